# revision 10
# baseline (speedup 1.0000x reference)
"""Distributed Trainium2 Bass kernel for causal multi-head attention (RoPE).

Reference computation (B=2, S=2048, D=2048, H=16, hd=128):
    q/k/v = x @ w{q,k,v}.T ; rope(q, k) ; causal softmax attention ; out @ wo.T

Sharding over 8 NeuronCores (tensor-parallel over heads, then rows):
  - Each core owns 2 heads: computes its q/k/v projections (256 features),
    RoPE, and causal attention for those heads.
  - Attention outputs (normalized by the softmax denominator via a broadcast
    trick) are exchanged with one AllToAll per local head so each core ends
    up with ALL features for 1/8 of the token rows; the per-head split lets
    the first collective overlap the second head's attention compute.
  - Each core computes its 512 rows of the output projection; the host
    concatenates the 8 row-chunks.

Everything is computed in bf16 on the TensorEngine with f32 PSUM
accumulation; softmax runs without max-subtraction (scores are O(1) by
construction) with the causal mask applied as a 0/1 multiply after exp.

Key engine-balance tricks (v2):
  - RoPE pair-swap uses the DVE STREAM_SHUFFLE partition permutation (one
    vector op) instead of a permutation-matrix matmul; rope multiplies run
    in bf16 on the vector engine.
  - Both batches' score blocks land in one 2-bank PSUM tile so each softmax
    exp is a single N=1024 ACTIVATE (halves the ACT instruction overhead
    that bounds the attention phase).
  - Softmax denominators are partition-all-reduced on the (otherwise idle)
    GPSIMD engine, which also broadcasts Z to all partitions — no PE matmuls
    in the normalization path at all.
  - Causal structure: fully-masked j-blocks are skipped; on the 4 diagonal
    j-blocks of each 512-wide i-chunk only the live suffix of queries is
    computed, so just one triangular 128x128 corner needs the 0/1 mask.
  - DMA queue plan: sync carries x tiles + wq (interleaved per k-tile for a
    fast start) + collective staging; the scalar HWDGE queue carries
    wk/wv/wo; gpsimd carries only the rope tables early and stays free so
    the AllToAll triggers are not delayed.
  - The output projection runs in two phases (even k-tiles, then odd) with
    partial sums parked in SBUF, so a full pass of matmuls is available to
    overlap the second AllToAll.
"""

import numpy as np
import ml_dtypes

import concourse.mybir as mybir
import concourse.tile as tile
from concourse import bacc, bass_isa
from concourse.bass_utils import run_bass_kernel_spmd

# Problem constants (hardcoded per harness contract)
B, S, D, H = 2, 2048, 2048, 16
W = 8  # cores
N = B * S  # 4096 tokens
HD = D // H  # 128 head dim
HL = H // W  # 2 heads per core
DL = HL * HD  # 256 features per core
CH = 512  # token chunk
NCH = N // CH  # 8 chunks
KT = D // 128  # 16 contraction tiles
RPC = N // W  # 512 rows per core for the output projection
NVB = N // 128  # 32 v token-blocks
SB = S // CH  # 4 i-chunks per batch

F32 = mybir.dt.float32
BF16 = mybir.dt.bfloat16
MUL = mybir.AluOpType.mult
ADD = mybir.AluOpType.add

SWAP32 = [i ^ 1 for i in range(32)]  # within-pair partition swap for RoPE


def build_nc():
    nc = bacc.Bacc("TRN2", target_bir_lowering=False, debug=False, num_devices=W)

    xT = nc.dram_tensor("xT", [D, N], BF16, kind="ExternalInput").ap()
    wqT = nc.dram_tensor("wqT", [D, DL], BF16, kind="ExternalInput").ap()
    wkT = nc.dram_tensor("wkT", [D, DL], BF16, kind="ExternalInput").ap()
    wvT = nc.dram_tensor("wvT", [D, DL], BF16, kind="ExternalInput").ap()
    woT = nc.dram_tensor("woT", [D, D], BF16, kind="ExternalInput").ap()
    fc2 = nc.dram_tensor("fc2", [HD, N], BF16, kind="ExternalInput").ap()
    fss = nc.dram_tensor("fss", [HD, N], BF16, kind="ExternalInput").ap()
    mask2 = nc.dram_tensor("mask2", [128, 2, 128], BF16, kind="ExternalInput").ap()
    out = nc.dram_tensor("out", [RPC, D], F32, kind="ExternalOutput").ap()

    with tile.TileContext(nc) as tc:
        _body(tc, xT, wqT, wkT, wvT, woT, fc2, fss, mask2, out)

    nc.compile()
    return nc


def _body(tc, xT, wqT, wkT, wvT, woT, fc2, fss, mask2, out):
    nc = tc.nc
    EXP = mybir.ActivationFunctionType.Exp

    with (
        tc.tile_pool(name="const", bufs=1) as const,
        tc.tile_pool(name="dram", bufs=1, space="DRAM") as dram,
    ):
        # ---- persistent SBUF state ----
        # wq streams per k-tile on the sync queue interleaved with chunk-0 x
        # tiles (the first matmuls need matching (x, wq) slices, nothing
        # more); wk/wv stream per k-tile on the scalar HWDGE queue.
        wq_sb, wk_sb, wv_sb = {}, {}, {}
        for half in range(2):
            for d, nm in ((wq_sb, "q"), (wk_sb, "k"), (wv_sb, "v")):
                d[half] = const.tile([128, 8, DL], BF16, name=f"w{nm}_h{half}")

        def w_slice_load(eng, t, w_src, half, k):
            kt = half * 8 + k
            eng.dma_start(t[:, k, :], w_src[kt * 128 : (kt + 1) * 128, :])

        # rope tables + mask early on the gpsimd queue (free until the
        # collectives fire at the end of attention)
        fc2_sb = const.tile([128, N], BF16)
        fss_sb = const.tile([128, N], BF16)
        for part in range(4):
            tsl = slice(part * (N // 4), (part + 1) * (N // 4))
            nc.gpsimd.dma_start(fc2_sb[:, tsl], fc2[:, tsl])
            nc.gpsimd.dma_start(fss_sb[:, tsl], fss[:, tsl])
        mask_sb = const.tile([128, 2, 128], BF16)
        nc.gpsimd.dma_start(mask_sb[:], mask2)

        # wk/wv per-slice on the scalar queue, in first-use order
        for k in range(8):
            w_slice_load(nc.scalar, wk_sb[0], wkT, 0, k)
        for k in range(8):
            w_slice_load(nc.scalar, wk_sb[1], wkT, 1, k)
        for k in range(8):
            w_slice_load(nc.scalar, wv_sb[0], wvT, 0, k)
        for k in range(8):
            w_slice_load(nc.scalar, wv_sb[1], wvT, 1, k)

        qT_sb = const.tile([128, HL, N], BF16)  # feature-major q (post-rope)
        kT_sb = const.tile([128, HL, N], BF16)
        v_sb = const.tile([128, NVB, DL], BF16)  # token-major v
        # post-A2A row tiles, feature-major; one tile per k-tile so phase-A
        # matmuls only depend on the first AllToAll's DMAs
        attn_t = [
            const.tile([128, CH], BF16, name=f"attn_t{kt}") for kt in range(KT)
        ]

        # per-head A2A buffers (shard s of head h = oT for rows [512s, 512s+512))
        a2a_in = [dram.tile([W, HD, CH], BF16, name=f"a2a_in{h}") for h in range(HL)]
        a2a_out = [dram.tile([W, HD, CH], BF16, name=f"a2a_out{h}") for h in range(HL)]

        # ================= stage 1: q/k/v projections + RoPE =================
        # K-contiguous per output tensor: all q matmuls for a chunk, then all
        # k, then all v (x tiles stay cached in SBUF).  Each tensor's PSUM
        # eviction then overlaps the next tensor's matmul phase, so chunk
        # boundaries don't stall the TensorEngine.
        with (
            tc.tile_pool(name="xin", bufs=24) as xin_pool,
            tc.tile_pool(name="ev", bufs=4) as ev_pool,
            tc.tile_pool(name="ps1", bufs=1, space="PSUM") as ps1,
        ):
            def rope_evict(ps_t, sub, dst, tok):
                # dst = q*cos + pairswap(q)*sin, all bf16 on ACT+DVE
                tmp = ev_pool.tile([128, CH], BF16, tag="tmp")
                nc.scalar.copy(tmp[:], ps_t[:])  # frees the PSUM bank
                sw = ev_pool.tile([128, CH], BF16, tag="sw")
                nc.vector.stream_shuffle(sw[:], tmp[:], SWAP32)
                t1 = ev_pool.tile([128, CH], BF16, tag="t1")
                t2 = ev_pool.tile([128, CH], BF16, tag="t2")
                nc.vector.tensor_tensor(t1[:], tmp[:], fc2_sb[:, tok], MUL)
                nc.vector.tensor_tensor(t2[:], sw[:], fss_sb[:, tok], MUL)
                nc.vector.tensor_tensor(dst[:, sub, tok], t1[:], t2[:], ADD)

            for ch in range(NCH):
                tok = slice(ch * CH, (ch + 1) * CH)
                xts = []
                for kt in range(KT):
                    xt = xin_pool.tile([128, CH], BF16, tag="xt", name=f"xt{kt}")
                    nc.sync.dma_start(xt[:], xT[kt * 128 : (kt + 1) * 128, tok])
                    xts.append(xt)
                    if ch == 0:
                        # pace wq with the x tiles chunk 0 actually consumes
                        w_slice_load(nc.sync, wq_sb[kt // 8], wqT, kt // 8, kt % 8)
                ps_q = [
                    ps1.tile([128, CH], F32, tag=f"pq{s}", name=f"ps_q{s}")
                    for s in range(2)
                ]
                ps_k = [
                    ps1.tile([128, CH], F32, tag=f"pk{s}", name=f"ps_k{s}")
                    for s in range(2)
                ]
                ps_v = [
                    ps1.tile([128, 2, 256], F32, tag=f"pv{s}", name=f"ps_v{s}")
                    for s in range(2)
                ]
                for kt in range(KT):
                    st, sp = kt == 0, kt == KT - 1
                    for sub in range(2):
                        fsl = slice(sub * 128, (sub + 1) * 128)
                        nc.tensor.matmul(
                            ps_q[sub][:], wq_sb[kt // 8][:, kt % 8, fsl], xts[kt][:],
                            start=st, stop=sp,
                        )
                for sub in range(2):
                    rope_evict(ps_q[sub], sub, qT_sb, tok)
                for kt in range(KT):
                    st, sp = kt == 0, kt == KT - 1
                    for sub in range(2):
                        fsl = slice(sub * 128, (sub + 1) * 128)
                        nc.tensor.matmul(
                            ps_k[sub][:], wk_sb[kt // 8][:, kt % 8, fsl], xts[kt][:],
                            start=st, stop=sp,
                        )
                for sub in range(2):
                    rope_evict(ps_k[sub], sub, kT_sb, tok)
                for kt in range(KT):
                    st, sp = kt == 0, kt == KT - 1
                    for t in range(4):
                        # start=True zeroes the whole 2KB PSUM bank, so only
                        # the bank's first slice may set it (kt==0, even t)
                        nc.tensor.matmul(
                            ps_v[t // 2][:, t % 2, :],
                            xts[kt][:, t * 128 : (t + 1) * 128],
                            wv_sb[kt // 8][:, kt % 8, :],
                            start=(st and t % 2 == 0),
                            stop=sp,
                        )
                # evict v (token-major)
                for half in range(2):
                    nc.scalar.copy(
                        v_sb[:, ch * 4 + half * 2 : ch * 4 + half * 2 + 2, :],
                        ps_v[half][:],
                    )

        # preload ALL wo tiles on the scalar HWDGE queue so they stream in
        # during attention without delaying sync-queue staging writes or the
        # gpsimd-queue collective triggers
        with tc.tile_pool(name="wo", bufs=64) as wo_pool:
            wts = {}
            for n in range(D // CH):
                for kt in range(KT):
                    wt = wo_pool.tile([128, CH], BF16, tag="wo", name=f"wt{n}_{kt}")
                    nc.scalar.dma_start(
                        wt[:], woT[kt * 128 : (kt + 1) * 128, n * CH : (n + 1) * CH]
                    )
                    wts[(n, kt)] = wt

            # ================= stage 2: causal attention (head-outer) =========
            with (
                tc.tile_pool(name="pt", bufs=5) as pt_pool,
                tc.tile_pool(name="zv", bufs=3) as zv_pool,
                tc.tile_pool(name="ot", bufs=2) as ot_pool,
                tc.tile_pool(name="ps2", bufs=2, space="PSUM") as ps2,
            ):
                def emit_norm(h, ci, ps_os, zv):
                    # Z = partition-ALL-reduce of zv on GPSIMD (result is
                    # broadcast to every partition, so no PE matmuls needed)
                    zb = ot_pool.tile([128, 2, CH], F32, tag="zb")
                    nc.gpsimd.partition_all_reduce(
                        zb[:], zv[:], 128, bass_isa.ReduceOp.add
                    )
                    rz = ot_pool.tile([128, 2, CH], F32, tag="rz")
                    nc.vector.reciprocal_approx_fast(rz[:], zb[:])
                    for b in range(B):
                        otn = ot_pool.tile([128, CH], BF16, tag="otn")
                        nc.vector.tensor_tensor(otn[:], ps_os[b][:], rz[:, b, :], MUL)
                        nc.sync.dma_start(a2a_in[h][b * SB + ci, :, :], otn[:])

                def emit_pv(jb, pt, off, wid, ps_os, njb, h):
                    for b in range(B):
                        vb = b * (S // 128) + jb
                        nc.tensor.matmul(
                            ps_os[b][:, off:],
                            v_sb[:, vb, h * 128 : (h + 1) * 128],
                            pt[:, b, :wid],
                            start=(jb == 0),
                            stop=(jb == njb - 1),
                        )

                for h in range(HL):
                    # Both batches' score blocks share one 2-bank PSUM tile
                    # so exp is a single N=1024 ACTIVATE.  pv matmuls run a
                    # few steps behind via `pend` so the PE never waits on
                    # the ACT exp chain; Z-normalization is deferred into the
                    # following group.
                    pend = []
                    norm_q = []
                    norm_delay = 0
                    for ci in range(SB):
                        njb = 4 * ci + 4
                        ps_os = [
                            ps2.tile([128, CH], F32, tag="po", bufs=4, name=f"po{b}")
                            for b in range(B)
                        ]
                        zv = zv_pool.tile([128, 2, CH], BF16, tag="zv", bufs=3)
                        for jb in range(njb):
                            r = jb - 4 * ci  # diag position (>=0 on diagonal)
                            off = 128 * r if r > 0 else 0  # live query suffix
                            wid = CH - off
                            ps_s = ps2.tile([128, 2, CH], F32, tag="ps", bufs=2)
                            for b in range(B):
                                tok_i0 = b * S + ci * CH
                                tok_j = slice(b * S + jb * 128, b * S + (jb + 1) * 128)
                                nc.tensor.matmul(
                                    ps_s[:, b, :wid],
                                    kT_sb[:, h, tok_j],
                                    qT_sb[:, h, tok_i0 + off : tok_i0 + CH],
                                    start=True,
                                    stop=True,
                                )
                            pt = pt_pool.tile([128, 2, CH], BF16, tag="pt")
                            nc.scalar.activation(
                                pt[:, :, :wid], ps_s[:, :, :wid], EXP
                            )
                            if r >= 0:
                                # triangular corner: queries [128r, 128r+128)
                                nc.vector.tensor_tensor(
                                    pt[:, :, :128], pt[:, :, :128], mask_sb[:], MUL
                                )
                            if jb == 0:
                                nc.vector.tensor_copy(zv[:], pt[:])
                            else:
                                nc.vector.tensor_tensor(
                                    zv[:, :, off:],
                                    zv[:, :, off:],
                                    pt[:, :, :wid],
                                    ADD,
                                )
                            pend.append((jb, pt, off, wid, ps_os, njb, h))
                            if len(pend) > 2:
                                emit_pv(*pend.pop(0))
                            if norm_q:
                                norm_delay -= 1
                                if norm_delay <= 0:
                                    emit_norm(*norm_q.pop(0))
                                    norm_delay = 2
                        # drain pending norms if next group is too short
                        while norm_q:
                            emit_norm(*norm_q.pop(0))
                        norm_q.append((h, ci, ps_os, zv))
                        norm_delay = 2  # in jb-pair steps; pend depth 2
                    while pend:
                        emit_pv(*pend.pop(0))
                    while norm_q:
                        emit_norm(*norm_q.pop(0))

                    # ---- per-head AllToAll: head 0's collective overlaps head
                    # 1's attention compute; head 1's overlaps phase A below
                    nc.gpsimd.collective_compute(
                        "AllToAll",
                        mybir.AluOpType.bypass,
                        replica_groups=[list(range(W))],
                        ins=[a2a_in[h].opt()],
                        outs=[a2a_out[h].opt()],
                    )
                    # pull this head's row tiles into SBUF right away
                    src = a2a_out[h][:].rearrange("w d c -> (w d) c")
                    for blk in range(W):
                        nc.sync.dma_start(
                            attn_t[2 * blk + h][:], src[blk * 128 : (blk + 1) * 128, :]
                        )

            # ============ stage 4: output projection for this core's rows =====
            # Two phases so ALL even-k (head-0) matmuls can run while the
            # second AllToAll is still in flight: phase A accumulates even
            # k-tiles for every (n, m) output group and parks the partial sums
            # in SBUF; phase B adds the odd k-tiles and writes out.
            with (
                tc.tile_pool(name="par", bufs=16) as par_pool,
                tc.tile_pool(name="oev", bufs=3) as oev_pool,
                tc.tile_pool(name="ps4", bufs=4, space="PSUM") as ps4,
            ):
                NCHUNK = D // CH
                MS = RPC // 128
                partial = {}
                for n in range(NCHUNK):
                    for m in range(MS):
                        ps_out = ps4.tile([128, CH], F32, tag="pout", name="ps_outA")
                        for i, kt in enumerate(range(0, KT, 2)):
                            nc.tensor.matmul(
                                ps_out[:],
                                attn_t[kt][:, m * 128 : (m + 1) * 128],
                                wts[(n, kt)][:],
                                start=(i == 0),
                                stop=(i == KT // 2 - 1),
                            )
                        par = par_pool.tile(
                            [128, CH], F32, tag="par", name=f"par{n}_{m}"
                        )
                        nc.vector.tensor_copy(par[:], ps_out[:])
                        partial[(n, m)] = par
                for n in range(NCHUNK):
                    for m in range(MS):
                        ps_out = ps4.tile([128, CH], F32, tag="pout", name="ps_outB")
                        for i, kt in enumerate(range(1, KT, 2)):
                            nc.tensor.matmul(
                                ps_out[:],
                                attn_t[kt][:, m * 128 : (m + 1) * 128],
                                wts[(n, kt)][:],
                                start=(i == 0),
                                stop=(i == KT // 2 - 1),
                            )
                        oev = oev_pool.tile([128, CH], F32, tag="oev")
                        nc.vector.tensor_tensor(
                            oev[:], ps_out[:], partial[(n, m)][:], ADD
                        )
                        eng = nc.sync if (n * MS + m) % 2 == 0 else nc.scalar
                        eng.dma_start(
                            out[m * 128 : (m + 1) * 128, n * CH : (n + 1) * CH],
                            oev[:],
                        )


_NC_CACHE = None


def _get_nc():
    global _NC_CACHE
    if _NC_CACHE is None:
        _NC_CACHE = build_nc()
    return _NC_CACHE


def _prep_inputs(x, freq_cos, freq_sin, wq, wk, wv, wo):
    bf = ml_dtypes.bfloat16
    x = np.asarray(x, np.float32).reshape(N, D)
    fc = np.asarray(freq_cos, np.float32)  # [S, 64]
    fs = np.asarray(freq_sin, np.float32)
    wq = np.asarray(wq, np.float32)
    wk = np.asarray(wk, np.float32)
    wv = np.asarray(wv, np.float32)
    wo = np.asarray(wo, np.float32)

    xT = np.ascontiguousarray(x.T).astype(bf)  # [D, N]
    woT = np.ascontiguousarray(wo.T).astype(bf)  # [D, D]

    # RoPE tables, expanded to the full head dim and tiled over batch.
    # fc2[d, b*S+i] = cos(freq[i, d//2]); fss carries sin with the sign of the
    # pair-swap term: -sin for even d, +sin for odd d.
    fc2 = np.tile(np.repeat(fc.T, 2, axis=0), (1, B)).astype(np.float32)
    sgn = np.where(np.arange(HD) % 2 == 0, -1.0, 1.0).astype(np.float32)[:, None]
    fss = (np.tile(np.repeat(fs.T, 2, axis=0), (1, B)) * sgn).astype(np.float32)
    fc2 = np.ascontiguousarray(fc2).astype(bf)
    fss = np.ascontiguousarray(fss).astype(bf)

    # triangular 0/1 mask for the 128x128 diagonal corner: allow j <= i,
    # duplicated for the two batch planes of the merged pt tile
    jp = np.arange(128)[:, None]
    ii = np.arange(128)[None, :]
    mask01 = (jp <= ii).astype(np.float32)
    mask2 = np.ascontiguousarray(
        np.stack([mask01, mask01], axis=1)
    ).astype(bf)  # [128, 2, 128]

    scale = 1.0 / np.sqrt(HD)
    in_maps = []
    for c in range(W):
        rows = slice(c * DL, (c + 1) * DL)
        wqT = np.ascontiguousarray((wq[rows] * scale).T).astype(bf)
        wkT = np.ascontiguousarray(wk[rows].T).astype(bf)
        wvT = np.ascontiguousarray(wv[rows].T).astype(bf)
        in_maps.append(
            {
                "xT": xT,
                "wqT": wqT,
                "wkT": wkT,
                "wvT": wvT,
                "woT": woT,
                "fc2": fc2,
                "fss": fss,
                "mask2": mask2,
            }
        )
    return in_maps


def kernel(x, freq_cos, freq_sin, wq, wk, wv, wo, _trace=False, _trace_kwargs=None):
    nc = _get_nc()
    in_maps = _prep_inputs(x, freq_cos, freq_sin, wq, wk, wv, wo)
    kwargs = {}
    if _trace:
        kwargs.update(trace=True, **(_trace_kwargs or {}))
    res = run_bass_kernel_spmd(nc, in_maps, core_ids=list(range(W)), **kwargs)
    kernel.last_result = res
    full = np.concatenate([res.results[c]["out"] for c in range(W)], axis=0)
    return full.reshape(B, S, D).astype(np.float32)


# revision 17
# speedup vs baseline: 1.1610x; 1.1610x over previous
"""Distributed Trainium2 Bass kernel for causal multi-head attention (RoPE).

Reference computation (B=2, S=2048, D=2048, H=16, hd=128):
    q/k/v = x @ w{q,k,v}.T ; rope(q, k) ; causal softmax attention ; out @ wo.T

Sharding over 8 NeuronCores (tensor-parallel over heads, then rows):
  - Each core owns 2 heads: computes its q/k/v projections (256 features),
    RoPE, and causal attention for those heads.
  - Attention outputs (normalized by the softmax denominator via a broadcast
    trick) are exchanged with one AllToAll per local head so each core ends
    up with ALL features for 1/8 of the token rows; the per-head split lets
    the first collective overlap the second head's attention compute.
  - Each core computes its 512 rows of the output projection; the host
    concatenates the 8 row-chunks.

Everything is computed in bf16 on the TensorEngine with f32 PSUM
accumulation; softmax runs without max-subtraction (scores are O(1) by
construction) with the causal mask applied as a 0/1 multiply after exp.

Key engine-balance tricks:
  - RoPE pair-swap uses the DVE STREAM_SHUFFLE partition permutation (one
    vector op) instead of a permutation-matrix matmul; rope multiplies run
    in bf16 on the vector engine.
  - Both batches' score blocks land in one 2-bank PSUM tile so each softmax
    exp is a single N=1024 ACTIVATE (halves the ACT instruction overhead
    that bounds the attention phase).
  - Softmax denominators: partition-sum and 1/Z partition-broadcast via tiny
    matmuls through a single shared PSUM bank (zv is bf16 so it feeds the
    sum matmul directly, no staging copy).
  - Causal structure: fully-masked j-blocks are skipped; on the 4 diagonal
    j-blocks of each 512-wide i-chunk only the live suffix of queries is
    computed, so just one triangular 128x128 corner needs the 0/1 mask.
  - All bulk DMA runs on the sync HWDGE queue (projection weights
    interleaved per k-tile with chunk-0 x tiles; wo tiles trickled through
    chunks 1-7).  A DMA trigger emitted on an engine queue blocks that
    queue for the whole transfer, so the ACT queue only ever executes
    compute and the gpsimd queue stays free for the AllToAll triggers.
  - The output projection runs in two phases (even k-tiles, then odd) with
    partial sums parked in SBUF (bf16), so a full pass of matmuls is
    available to overlap the second AllToAll.
"""

import numpy as np
import ml_dtypes

import concourse.mybir as mybir
import concourse.tile as tile
from concourse import bacc
from concourse.bass_utils import run_bass_kernel_spmd

# Problem constants (hardcoded per harness contract)
B, S, D, H = 2, 2048, 2048, 16
W = 8  # cores
N = B * S  # 4096 tokens
HD = D // H  # 128 head dim
HL = H // W  # 2 heads per core
DL = HL * HD  # 256 features per core
CH = 512  # token chunk
NCH = N // CH  # 8 chunks
KT = D // 128  # 16 contraction tiles
RPC = N // W  # 512 rows per core for the output projection
NVB = N // 128  # 32 v token-blocks
SB = S // CH  # 4 i-chunks per batch

F32 = mybir.dt.float32
BF16 = mybir.dt.bfloat16
MUL = mybir.AluOpType.mult
ADD = mybir.AluOpType.add

SWAP32 = [i ^ 1 for i in range(32)]  # within-pair partition swap for RoPE


def build_nc():
    nc = bacc.Bacc("TRN2", target_bir_lowering=False, debug=False, num_devices=W)

    xT = nc.dram_tensor("xT", [D, N], BF16, kind="ExternalInput").ap()
    wqT = nc.dram_tensor("wqT", [D, DL], BF16, kind="ExternalInput").ap()
    wkT = nc.dram_tensor("wkT", [D, DL], BF16, kind="ExternalInput").ap()
    wvT = nc.dram_tensor("wvT", [D, DL], BF16, kind="ExternalInput").ap()
    woT = nc.dram_tensor("woT", [D, D], BF16, kind="ExternalInput").ap()
    fc2 = nc.dram_tensor("fc2", [HD, N], BF16, kind="ExternalInput").ap()
    fss = nc.dram_tensor("fss", [HD, N], BF16, kind="ExternalInput").ap()
    mask2 = nc.dram_tensor("mask2", [128, 2, 128], BF16, kind="ExternalInput").ap()
    out = nc.dram_tensor("out", [RPC, D], F32, kind="ExternalOutput").ap()

    with tile.TileContext(nc) as tc:
        _body(tc, xT, wqT, wkT, wvT, woT, fc2, fss, mask2, out)

    nc.compile()
    return nc


def _body(tc, xT, wqT, wkT, wvT, woT, fc2, fss, mask2, out):
    nc = tc.nc
    EXP = mybir.ActivationFunctionType.Exp

    with (
        tc.tile_pool(name="const", bufs=1) as const,
        tc.tile_pool(name="dram", bufs=1, space="DRAM") as dram,
        tc.tile_pool(name="wo", bufs=64) as wo_pool,
    ):
        # ---- persistent SBUF state ----
        wq_sb, wk_sb, wv_sb = {}, {}, {}
        for half in range(2):
            for d, nm in ((wq_sb, "q"), (wk_sb, "k"), (wv_sb, "v")):
                d[half] = const.tile([128, 8, DL], BF16, name=f"w{nm}_h{half}")

        def w_slice_load(t, w_src, half, k):
            kt = half * 8 + k
            nc.sync.dma_start(t[:, k, :], w_src[kt * 128 : (kt + 1) * 128, :])

        # rope tables + mask early on the gpsimd queue (free until the
        # collectives fire at the end of attention)
        fc2_sb = const.tile([128, N], BF16)
        fss_sb = const.tile([128, N], BF16)
        for part in range(4):
            tsl = slice(part * (N // 4), (part + 1) * (N // 4))
            nc.gpsimd.dma_start(fc2_sb[:, tsl], fc2[:, tsl])
            nc.gpsimd.dma_start(fss_sb[:, tsl], fss[:, tsl])
        mask_sb = const.tile([128, 2, 128], BF16)
        nc.gpsimd.dma_start(mask_sb[:], mask2)
        ones_col = const.tile([128, 1], BF16)
        nc.vector.memset(ones_col[:], 1.0)
        ones_row = const.tile([1, 128], BF16)
        nc.vector.memset(ones_row[:], 1.0)

        qT_sb = const.tile([128, HL, N], BF16)  # feature-major q (post-rope)
        kT_sb = const.tile([128, HL, N], BF16)
        v_sb = const.tile([128, NVB, DL], BF16)  # token-major v
        # post-A2A row tiles, feature-major; one tile per k-tile so phase-A
        # matmuls only depend on the first AllToAll's DMAs
        attn_t = [
            const.tile([128, CH], BF16, name=f"attn_t{kt}") for kt in range(KT)
        ]

        # per-head A2A buffers (shard s of head h = oT for rows [512s, 512s+512))
        a2a_in = [dram.tile([W, HD, CH], BF16, name=f"a2a_in{h}") for h in range(HL)]
        a2a_out = [dram.tile([W, HD, CH], BF16, name=f"a2a_out{h}") for h in range(HL)]

        # wo tiles: allocated up front; their loads trickle through the sync
        # queue during stage-1 chunks 1-7
        wts = {}
        wo_jobs = []
        for n in range(D // CH):
            for kt in range(KT):
                wts[(n, kt)] = wo_pool.tile(
                    [128, CH], BF16, tag="wo", name=f"wt{n}_{kt}"
                )
                wo_jobs.append((n, kt))

        # ================= stage 1: q/k/v projections + RoPE =================
        # K-contiguous per output tensor: all q matmuls for a chunk, then all
        # k, then all v (x tiles stay cached in SBUF).  Each tensor's PSUM
        # eviction then overlaps the next tensor's matmul phase, so chunk
        # boundaries don't stall the TensorEngine.
        with (
            tc.tile_pool(name="xin", bufs=20) as xin_pool,
            tc.tile_pool(name="ev", bufs=3) as ev_pool,
            tc.tile_pool(name="ps1", bufs=1, space="PSUM") as ps1,
        ):
            def rope_evict(ps_t, sub, dst, tok):
                # dst = q*cos + pairswap(q)*sin, all bf16 on ACT+DVE
                tmp = ev_pool.tile([128, CH], BF16, tag="tmp")
                nc.scalar.copy(tmp[:], ps_t[:])  # frees the PSUM bank
                sw = ev_pool.tile([128, CH], BF16, tag="sw")
                nc.vector.stream_shuffle(sw[:], tmp[:], SWAP32)
                t1 = ev_pool.tile([128, CH], BF16, tag="t1")
                t2 = ev_pool.tile([128, CH], BF16, tag="t2")
                nc.vector.tensor_tensor(t1[:], tmp[:], fc2_sb[:, tok], MUL)
                nc.vector.tensor_tensor(t2[:], sw[:], fss_sb[:, tok], MUL)
                nc.vector.tensor_tensor(dst[:, sub, tok], t1[:], t2[:], ADD)

            for ch in range(NCH):
                tok = slice(ch * CH, (ch + 1) * CH)
                xts = []
                for kt in range(KT):
                    xt = xin_pool.tile([128, CH], BF16, tag="xt", name=f"xt{kt}")
                    nc.sync.dma_start(xt[:], xT[kt * 128 : (kt + 1) * 128, tok])
                    xts.append(xt)
                    if ch == 0:
                        # pace all projection weights with the x tiles that
                        # chunk 0 consumes (everything on the one sync queue)
                        w_slice_load(wq_sb[kt // 8], wqT, kt // 8, kt % 8)
                        w_slice_load(wk_sb[kt // 8], wkT, kt // 8, kt % 8)
                        w_slice_load(wv_sb[kt // 8], wvT, kt // 8, kt % 8)
                if ch > 0:
                    for _ in range(10):
                        if wo_jobs:
                            n_, kt_ = wo_jobs.pop(0)
                            nc.sync.dma_start(
                                wts[(n_, kt_)][:],
                                woT[
                                    kt_ * 128 : (kt_ + 1) * 128,
                                    n_ * CH : (n_ + 1) * CH,
                                ],
                            )
                ps_q = [
                    ps1.tile([128, CH], F32, tag=f"pq{s}", name=f"ps_q{s}")
                    for s in range(2)
                ]
                ps_k = [
                    ps1.tile([128, CH], F32, tag=f"pk{s}", name=f"ps_k{s}")
                    for s in range(2)
                ]
                ps_v = [
                    ps1.tile([128, 2, 256], F32, tag=f"pv{s}", name=f"ps_v{s}")
                    for s in range(2)
                ]
                for kt in range(KT):
                    st, sp = kt == 0, kt == KT - 1
                    for sub in range(2):
                        fsl = slice(sub * 128, (sub + 1) * 128)
                        nc.tensor.matmul(
                            ps_q[sub][:], wq_sb[kt // 8][:, kt % 8, fsl], xts[kt][:],
                            start=st, stop=sp,
                        )
                for sub in range(2):
                    rope_evict(ps_q[sub], sub, qT_sb, tok)
                for kt in range(KT):
                    st, sp = kt == 0, kt == KT - 1
                    for sub in range(2):
                        fsl = slice(sub * 128, (sub + 1) * 128)
                        nc.tensor.matmul(
                            ps_k[sub][:], wk_sb[kt // 8][:, kt % 8, fsl], xts[kt][:],
                            start=st, stop=sp,
                        )
                for sub in range(2):
                    rope_evict(ps_k[sub], sub, kT_sb, tok)
                for kt in range(KT):
                    st, sp = kt == 0, kt == KT - 1
                    for t in range(4):
                        # start=True zeroes the whole 2KB PSUM bank, so only
                        # the bank's first slice may set it (kt==0, even t)
                        nc.tensor.matmul(
                            ps_v[t // 2][:, t % 2, :],
                            xts[kt][:, t * 128 : (t + 1) * 128],
                            wv_sb[kt // 8][:, kt % 8, :],
                            start=(st and t % 2 == 0),
                            stop=sp,
                        )
                # evict v (token-major)
                for half in range(2):
                    nc.scalar.copy(
                        v_sb[:, ch * 4 + half * 2 : ch * 4 + half * 2 + 2, :],
                        ps_v[half][:],
                    )

        # ================= stage 2: causal attention (head-outer) =========
        with (
            tc.tile_pool(name="pt", bufs=5) as pt_pool,
            tc.tile_pool(name="zv", bufs=3) as zv_pool,
            tc.tile_pool(name="ot", bufs=2) as ot_pool,
            tc.tile_pool(name="ps2", bufs=2, space="PSUM") as ps2,
        ):
            def emit_norm(h, ci, ps_os, zv):
                # Z = partition-sum of zv (bf16, direct moving operand) and
                # 1/Z partition-broadcast, both through ONE shared PSUM bank
                rz = ot_pool.tile([1, 2, CH], F32, tag="rz")
                rzb = ot_pool.tile([1, 2, CH], BF16, tag="rzb")
                for b in range(B):
                    pn = ps2.tile([128, CH], F32, tag="pn", bufs=1)
                    nc.tensor.matmul(
                        pn[0:1, :], ones_col[:], zv[:, b, :], start=True, stop=True
                    )
                    nc.vector.reciprocal_approx_fast(rz[:, b, :], pn[0:1, :])
                nc.vector.tensor_copy(rzb[:], rz[:])
                for b in range(B):
                    pn = ps2.tile([128, CH], F32, tag="pn", bufs=1)
                    nc.tensor.matmul(
                        pn[:], ones_row[:], rzb[:, b, :], start=True, stop=True
                    )
                    bc = ot_pool.tile([128, CH], F32, tag="bc")
                    nc.vector.tensor_copy(bc[:], pn[:])
                    otn = ot_pool.tile([128, CH], BF16, tag="otn")
                    nc.vector.tensor_tensor(otn[:], ps_os[b][:], bc[:], MUL)
                    nc.sync.dma_start(a2a_in[h][b * SB + ci, :, :], otn[:])

            def emit_pv(jb, pt, off, wid, ps_os, njb, h):
                for b in range(B):
                    vb = b * (S // 128) + jb
                    nc.tensor.matmul(
                        ps_os[b][:, off:],
                        v_sb[:, vb, h * 128 : (h + 1) * 128],
                        pt[:, b, :wid],
                        start=(jb == 0),
                        stop=(jb == njb - 1),
                    )

            for h in range(HL):
                # Both batches' score blocks share one 2-bank PSUM tile
                # so exp is a single N=1024 ACTIVATE.  pv matmuls run a
                # few steps behind via `pend` so the PE never waits on
                # the ACT exp chain; Z-normalization is deferred into the
                # following group.
                pend = []
                norm_q = []
                norm_delay = 0
                for ci in range(SB):
                    njb = 4 * ci + 4
                    ps_os = [
                        ps2.tile([128, CH], F32, tag="po", bufs=3, name=f"po{b}")
                        for b in range(B)
                    ]
                    zv = zv_pool.tile([128, 2, CH], BF16, tag="zv", bufs=3)
                    for jb in range(njb):
                        r = jb - 4 * ci  # diag position (>=0 on diagonal)
                        off = 128 * r if r > 0 else 0  # live query suffix
                        wid = CH - off
                        ps_s = ps2.tile([128, 2, CH], F32, tag="ps", bufs=2)
                        for b in range(B):
                            tok_i0 = b * S + ci * CH
                            tok_j = slice(b * S + jb * 128, b * S + (jb + 1) * 128)
                            nc.tensor.matmul(
                                ps_s[:, b, :wid],
                                kT_sb[:, h, tok_j],
                                qT_sb[:, h, tok_i0 + off : tok_i0 + CH],
                                start=True,
                                stop=True,
                            )
                        pt = pt_pool.tile([128, 2, CH], BF16, tag="pt")
                        nc.scalar.activation(
                            pt[:, :, :wid], ps_s[:, :, :wid], EXP
                        )
                        if r >= 0:
                            # triangular corner: queries [128r, 128r+128)
                            nc.vector.tensor_tensor(
                                pt[:, :, :128], pt[:, :, :128], mask_sb[:], MUL
                            )
                        if jb == 0:
                            nc.vector.tensor_scalar_add(zv[:], pt[:], 0.0)
                        else:
                            nc.vector.tensor_tensor(
                                zv[:, :, off:],
                                zv[:, :, off:],
                                pt[:, :, :wid],
                                ADD,
                            )
                        pend.append((jb, pt, off, wid, ps_os, njb, h))
                        if len(pend) > 2:
                            emit_pv(*pend.pop(0))
                        if norm_q:
                            norm_delay -= 1
                            if norm_delay <= 0:
                                emit_norm(*norm_q.pop(0))
                                norm_delay = 2
                    # drain pending norms if next group is too short
                    while norm_q:
                        emit_norm(*norm_q.pop(0))
                    norm_q.append((h, ci, ps_os, zv))
                    norm_delay = 2  # in jb-pair steps; pend depth 2
                while pend:
                    emit_pv(*pend.pop(0))
                while norm_q:
                    emit_norm(*norm_q.pop(0))

                # ---- per-head AllToAll: head 0's collective overlaps head
                # 1's attention compute; head 1's overlaps phase A below
                nc.gpsimd.collective_compute(
                    "AllToAll",
                    mybir.AluOpType.bypass,
                    replica_groups=[list(range(W))],
                    ins=[a2a_in[h].opt()],
                    outs=[a2a_out[h].opt()],
                )
                # pull this head's row tiles into SBUF right away
                src = a2a_out[h][:].rearrange("w d c -> (w d) c")
                for blk in range(W):
                    nc.sync.dma_start(
                        attn_t[2 * blk + h][:], src[blk * 128 : (blk + 1) * 128, :]
                    )

        # ============ stage 4: output projection for this core's rows =====
        # Two phases so ALL even-k (head-0) matmuls can run while the
        # second AllToAll is still in flight: phase A accumulates even
        # k-tiles for every (n, m) output group and parks the partial sums
        # in SBUF; phase B adds the odd k-tiles and writes out.
        with (
            tc.tile_pool(name="par", bufs=16) as par_pool,
            tc.tile_pool(name="oev", bufs=3) as oev_pool,
            tc.tile_pool(name="ps4", bufs=4, space="PSUM") as ps4,
        ):
            NCHUNK = D // CH
            MS = RPC // 128
            partial = {}
            for n in range(NCHUNK):
                for m in range(MS):
                    ps_out = ps4.tile([128, CH], F32, tag="pout", name="ps_outA")
                    for i, kt in enumerate(range(0, KT, 2)):
                        nc.tensor.matmul(
                            ps_out[:],
                            attn_t[kt][:, m * 128 : (m + 1) * 128],
                            wts[(n, kt)][:],
                            start=(i == 0),
                            stop=(i == KT // 2 - 1),
                        )
                    par = par_pool.tile(
                        [128, CH], BF16, tag="par", name=f"par{n}_{m}"
                    )
                    nc.vector.tensor_copy(par[:], ps_out[:])
                    partial[(n, m)] = par
            for n in range(NCHUNK):
                for m in range(MS):
                    ps_out = ps4.tile([128, CH], F32, tag="pout", name="ps_outB")
                    for i, kt in enumerate(range(1, KT, 2)):
                        nc.tensor.matmul(
                            ps_out[:],
                            attn_t[kt][:, m * 128 : (m + 1) * 128],
                            wts[(n, kt)][:],
                            start=(i == 0),
                            stop=(i == KT // 2 - 1),
                        )
                    oev = oev_pool.tile([128, CH], F32, tag="oev")
                    nc.vector.tensor_tensor(
                        oev[:], ps_out[:], partial[(n, m)][:], ADD
                    )
                    eng = nc.sync if (n * MS + m) % 2 == 0 else nc.scalar
                    eng.dma_start(
                        out[m * 128 : (m + 1) * 128, n * CH : (n + 1) * CH],
                        oev[:],
                    )


_NC_CACHE = None


def _get_nc():
    global _NC_CACHE
    if _NC_CACHE is None:
        _NC_CACHE = build_nc()
    return _NC_CACHE


def _prep_inputs(x, freq_cos, freq_sin, wq, wk, wv, wo):
    bf = ml_dtypes.bfloat16
    x = np.asarray(x, np.float32).reshape(N, D)
    fc = np.asarray(freq_cos, np.float32)  # [S, 64]
    fs = np.asarray(freq_sin, np.float32)
    wq = np.asarray(wq, np.float32)
    wk = np.asarray(wk, np.float32)
    wv = np.asarray(wv, np.float32)
    wo = np.asarray(wo, np.float32)

    xT = np.ascontiguousarray(x.T).astype(bf)  # [D, N]
    woT = np.ascontiguousarray(wo.T).astype(bf)  # [D, D]

    # RoPE tables, expanded to the full head dim and tiled over batch.
    # fc2[d, b*S+i] = cos(freq[i, d//2]); fss carries sin with the sign of the
    # pair-swap term: -sin for even d, +sin for odd d.
    fc2 = np.tile(np.repeat(fc.T, 2, axis=0), (1, B)).astype(np.float32)
    sgn = np.where(np.arange(HD) % 2 == 0, -1.0, 1.0).astype(np.float32)[:, None]
    fss = (np.tile(np.repeat(fs.T, 2, axis=0), (1, B)) * sgn).astype(np.float32)
    fc2 = np.ascontiguousarray(fc2).astype(bf)
    fss = np.ascontiguousarray(fss).astype(bf)

    # triangular 0/1 mask for the 128x128 diagonal corner: allow j <= i,
    # duplicated for the two batch planes of the merged pt tile
    jp = np.arange(128)[:, None]
    ii = np.arange(128)[None, :]
    mask01 = (jp <= ii).astype(np.float32)
    mask2 = np.ascontiguousarray(
        np.stack([mask01, mask01], axis=1)
    ).astype(bf)  # [128, 2, 128]

    scale = 1.0 / np.sqrt(HD)
    in_maps = []
    for c in range(W):
        rows = slice(c * DL, (c + 1) * DL)
        wqT = np.ascontiguousarray((wq[rows] * scale).T).astype(bf)
        wkT = np.ascontiguousarray(wk[rows].T).astype(bf)
        wvT = np.ascontiguousarray(wv[rows].T).astype(bf)
        in_maps.append(
            {
                "xT": xT,
                "wqT": wqT,
                "wkT": wkT,
                "wvT": wvT,
                "woT": woT,
                "fc2": fc2,
                "fss": fss,
                "mask2": mask2,
            }
        )
    return in_maps


def kernel(x, freq_cos, freq_sin, wq, wk, wv, wo, _trace=False, _trace_kwargs=None):
    nc = _get_nc()
    in_maps = _prep_inputs(x, freq_cos, freq_sin, wq, wk, wv, wo)
    kwargs = {}
    if _trace:
        kwargs.update(trace=True, **(_trace_kwargs or {}))
    res = run_bass_kernel_spmd(nc, in_maps, core_ids=list(range(W)), **kwargs)
    kernel.last_result = res
    full = np.concatenate([res.results[c]["out"] for c in range(W)], axis=0)
    return full.reshape(B, S, D).astype(np.float32)
